# revision 40
# baseline (speedup 1.0000x reference)
"""Trainium2 Bass kernel for Llama-style GQA attention (nn_LlamaAttention).

Shapes (hardcoded): hidden [1, 2048, 2048] f32, Wq [2048, 2048],
Wk/Wv [512, 2048], Wo [2048, 2048]. 32 q heads, 8 kv heads, head_dim 64,
causal + interleaved RoPE.

Sharding: tensor-parallel over heads across 8 NeuronCores. Core c owns
q heads 4c..4c+3 (one GQA group) and kv head c. Each core computes its
q/k/v projections (output-dim shard), RoPE, causal attention for its 4
heads, and a partial output projection (Wo input-dim shard). The host
sums the 8 partial [s, m] outputs.

All matmul operands are bf16 (true 1 PE cycle/row on TRN2 silicon vs
~2.5-3.7 measured for f32r) with f32 PSUM accumulation. RoPE runs in
f32 off PSUM and rounds once into the bf16 q/k tiles; P=exp(s) rounds
to bf16 going into the PV matmul; partial y is written back in bf16
(host sums in f64). Measured rel err vs the f32 reference ~4.5e-3,
under the 2e-2 gate.

On-core dataflow (everything "transposed", seq dim on the free axis):
  hT [h, s] -> qT [256, s], kT/vT [64, s]  (bf16 matmuls, N=512 chunks)
  RoPE via pair-swap (partition-strided SBUF-SBUF DMA) + DVE combine
  scores sT[j, i] = kT^T q, two heads packed in the PE array (K=64 row tiles)
  causal: lower-left block skipping; diagonal 128x128 blocks are masked
    on the PE itself (identity x (-1e9 upper-triangle) accumulated into
    the scores PSUM) so exp yields exact zeros with no DVE dependency
  P = exp(sT) on ScalarE (no max subtraction; scores are O(1) bounded)
  O^T accumulation with a ones-column in V to get the softmax denominator
  PSUM staged to SBUF immediately (frees banks for the next head pair);
    normalize later via DVE reciprocal + gpsimd partition-broadcast
  y[s, m] = O^T^T @ Wo_shard^T partials, summed on host.

Engine budget per core (~250us): PE ~100% busy (projections 98k rows,
scores+PV 2x70k rows at 64-deep contraction, out-proj 66k, masks 8k);
ScalarE ~75% (exp is a ~93us floor); DVE ~40%; DMA ~28MB.
"""

import numpy as np

HIDDEN = 2048
S = 2048
NH = 32
NKV = 8
HD = 64
GROUPS = 4
N_CORES = 8
DQ = 256          # q output dims per core (4 heads x 64)
CH = 512          # seq chunk width
NCH = S // CH     # 4
KT = HIDDEN // 128  # 16 contraction tiles

_cache = {}


def _build_program(repeat=1):
    import concourse.bacc as bacc
    import concourse.mybir as mybir
    import concourse.tile as tile

    f32 = mybir.dt.float32
    bf16 = mybir.dt.bfloat16
    EXP = mybir.ActivationFunctionType.Exp

    nc = bacc.Bacc("TRN2", target_bir_lowering=False, debug=False,
                   num_devices=N_CORES)

    hT = nc.declare_dram_parameter("hT", [HIDDEN, S], bf16, isOutput=False)
    wqkvT = nc.declare_dram_parameter("wqkvT", [HIDDEN, DQ + 2 * HD], bf16,
                                      isOutput=False)
    woT = nc.declare_dram_parameter("woT", [DQ, HIDDEN], bf16, isOutput=False)
    tables = nc.declare_dram_parameter("tables", [128, NCH, 4 * CH], f32,
                                       isOutput=False)
    trineg = nc.declare_dram_parameter("trineg", [128, 128], bf16,
                                       isOutput=False)
    ident128 = nc.declare_dram_parameter("ident128", [128, 128], bf16,
                                         isOutput=False)
    onespad = nc.declare_dram_parameter("onespad", [128, 64], bf16,
                                        isOutput=False)
    ident2 = nc.declare_dram_parameter("ident2", [128, 64], bf16,
                                       isOutput=False)
    y = nc.declare_dram_parameter("y", [S, HIDDEN], bf16, isOutput=True)

    with tile.TileContext(nc) as tc:
        with (
            tc.tile_pool(name="const", bufs=1) as const,
            tc.tile_pool(name="weights", bufs=1) as wpool,
            tc.tile_pool(name="ht", bufs=1) as htp,
            tc.tile_pool(name="work", bufs=1) as work,
            tc.tile_pool(name="persist", bufs=1) as persist,
            tc.tile_pool(name="vp", bufs=1) as vp,
            tc.tile_pool(name="ptp", bufs=1) as ptp,
            tc.tile_pool(name="ppa", bufs=1, space="PSUM") as ppa,
            tc.tile_pool(name="ppv", bufs=1, space="PSUM") as ppv,
            tc.tile_pool(name="pps", bufs=1, space="PSUM") as pps,
            tc.tile_pool(name="ppo", bufs=1, space="PSUM") as ppo,
        ):
            # ---- constants / weights ----
            t_trineg = const.tile([128, 128], bf16)
            t_id128 = const.tile([128, 128], bf16)
            t_ones = const.tile([128, 64], bf16)
            t_id = const.tile([128, 64], bf16)

            t_wqkv = wpool.tile([128, KT, DQ + 2 * HD], bf16)
            wqkv_r = wqkvT[:, :].rearrange("(t p) o -> p t o", p=128)

            # persistent activations
            t_q = [persist.tile([128, S], bf16, tag=f"q{m}", name=f"t_q{m}")
                   for m in range(2)]
            t_k = persist.tile([128, S], bf16, tag="k")
            t_ot = [persist.tile([128, S], bf16, tag=f"ot{m}", name=f"t_ot{m}")
                    for m in range(2)]
            t_v = [vp.tile([128, 192], bf16, tag=f"v{j}", name=f"t_v{j}")
                   for j in range(KT)]

            t_wo = None

            def rope_combine(dst, raw, swp, cos_t, sin_t, rows, ci):
                """dst[:, chunk] = raw*cos + swp*sin  (rows slice, bf16 out)."""
                r0, r1 = rows
                c0 = ci * CH
                tmp1 = work.tile([128, CH], f32, tag="rc1", bufs=4)
                tmp2 = work.tile([128, CH], f32, tag="rc2", bufs=4)
                nc.vector.tensor_mul(tmp1[r0:r1], raw[r0:r1],
                                     cos_t[r0:r1, :])
                nc.vector.tensor_mul(tmp2[r0:r1], swp[r0:r1],
                                     sin_t[r0:r1, :])
                nc.vector.tensor_add(dst[r0:r1, c0:c0 + CH], tmp1[r0:r1],
                                     tmp2[r0:r1])

            def swap_dma(dst, src, r0, r1):
                """dst[r0:r1] = src[r0:r1] with even/odd partition pairs swapped."""
                dv = dst[r0:r1, :].rearrange("(a two) s -> a two s", two=2)
                sv = src[r0:r1, :].rearrange("(a two) s -> a two s", two=2)
                nc.sync.dma_start(out=dv[:, 0, :], in_=sv[:, 1, :])
                nc.sync.dma_start(out=dv[:, 1, :], in_=sv[:, 0, :])

            hT_r = hT[:, :].rearrange("(t p) s -> p t s", p=128)

            def emit_A(ci):
                """Projections + RoPE + k/v prep for s-chunk ci."""
                c0 = ci * CH
                ht_t = htp.tile([128, KT, CH], bf16, tag="ht", bufs=2,
                                name=f"ht_{ci}")
                if ci == 0:
                    # split loads so the first kv matmul (kt=0) can start
                    # after ~200KB instead of the full 3.5MB; consts and
                    # the bulk follow behind
                    nc.sync.dma_start(out=t_wqkv[:, 0:1, :],
                                      in_=wqkv_r[:, 0:1, :])
                    nc.sync.dma_start(out=ht_t[:, 0:1, :],
                                      in_=hT_r[:, 0:1, c0:c0 + CH])
                    nc.sync.dma_start(out=t_wqkv[:, 1:8, :],
                                      in_=wqkv_r[:, 1:8, :])
                    nc.sync.dma_start(out=t_wqkv[:, 8:KT, :],
                                      in_=wqkv_r[:, 8:KT, :])
                    nc.sync.dma_start(out=ht_t[:, 1:8, :],
                                      in_=hT_r[:, 1:8, c0:c0 + CH])
                    nc.sync.dma_start(out=ht_t[:, 8:KT, :],
                                      in_=hT_r[:, 8:KT, c0:c0 + CH])
                    nc.sync.dma_start(out=t_id, in_=ident2[:, :])
                    nc.sync.dma_start(out=t_ones, in_=onespad[:, :])
                    nc.sync.dma_start(out=t_trineg, in_=trineg[:, :])
                    nc.sync.dma_start(out=t_id128, in_=ident128[:, :])
                else:
                    # chunk hT load in halves: the first 8 kt-tiles arrive
                    # in ~3us so projection matmuls start without waiting
                    # for the full 2MB (B(0) is short, A(1) comes up fast)
                    nc.sync.dma_start(out=ht_t[:, 0:8, :],
                                      in_=hT_r[:, 0:8, c0:c0 + CH])
                    nc.sync.dma_start(out=ht_t[:, 8:KT, :],
                                      in_=hT_r[:, 8:KT, c0:c0 + CH])
                # rope tables for this chunk: one fused DMA
                t_tab = work.tile([128, 4 * CH], f32, tag="tab", bufs=2)
                nc.sync.dma_start(out=t_tab, in_=tables[:, ci, :])
                t_cosk = t_tab[:, 0:CH]
                t_sink = t_tab[:, CH:2 * CH]
                t_cosq = t_tab[:, 2 * CH:3 * CH]
                t_sinq = t_tab[:, 3 * CH:4 * CH]

                # kv pass first so k/v prep overlaps the q passes
                ps_kv = ppa.tile([128, CH], f32, tag="proj", bufs=1)
                for kt in range(KT):
                    nc.tensor.matmul(ps_kv, t_wqkv[:, kt, 256:384],
                                     ht_t[:, kt, :],
                                     start=(kt == 0), stop=(kt == KT - 1))
                # ---- k: copy, duplicate to upper partitions, swap, RoPE ----
                k_raw = work.tile([128, CH], f32, tag="kraw", bufs=2)
                nc.scalar.copy(k_raw[0:64], ps_kv[0:64])
                nc.sync.dma_start(out=k_raw[64:128, :], in_=k_raw[0:64, :])
                k_swp = work.tile([128, CH], f32, tag="kswp", bufs=2)
                swap_dma(k_swp, k_raw, 0, 64)
                nc.sync.dma_start(out=k_swp[64:128, :], in_=k_swp[0:64, :])
                rope_combine(t_k, k_raw, k_swp, t_cosk, t_sink, (0, 128), ci)

                # ---- v: transpose [64, CH] -> 4 x [128, 64] tiles ----
                v_raw = work.tile([128, CH], bf16, tag="vraw", bufs=2)
                nc.scalar.copy(v_raw[64:128], ps_kv[64:128])
                for b in range(4):
                    jb = 4 * ci + b
                    ps_vt = ppv.tile([128, 64], bf16, tag="misc", bufs=2)
                    nc.tensor.transpose(
                        ps_vt, v_raw[64:128, b * 128:(b + 1) * 128],
                        t_id[64:128, :])
                    # v tile layout: [v(0:64) | ones,zeros(64:128) | v(128:192)]
                    nc.scalar.copy(t_v[jb][:, 0:64], ps_vt)
                    nc.scalar.copy(t_v[jb][:, 128:192], ps_vt)
                    nc.sync.dma_start(out=t_v[jb][:, 64:128], in_=t_ones)

                # ---- q passes + RoPE (2 partition tiles = 4 heads) ----
                for m in range(2):
                    ps_q = ppa.tile([128, CH], f32, tag="proj", bufs=1,
                                    name=f"ps_q{m}")
                    for kt in range(KT):
                        nc.tensor.matmul(
                            ps_q, t_wqkv[:, kt, m * 128:(m + 1) * 128],
                            ht_t[:, kt, :],
                            start=(kt == 0), stop=(kt == KT - 1))
                    q_raw = work.tile([128, CH], f32, tag="qraw", bufs=2)
                    nc.scalar.copy(q_raw, ps_q)
                    q_swp = work.tile([128, CH], f32, tag="qswp", bufs=2)
                    swap_dma(q_swp, q_raw, 0, 128)
                    rope_combine(t_q[m], q_raw, q_swp, t_cosq, t_sinq,
                                 (0, 128), ci)

            def emit_B_pair(ci, pair):
                """Attention for i-chunk ci, one packed head pair (2 heads)."""
                c0 = ci * CH
                if ci == 0:
                    # Wo halves load during attention (DMA slack window)
                    nc.sync.dma_start(
                        out=t_wo[:, pair, :],
                        in_=woT[:, :].rearrange(
                            "(t p) o -> p t o", p=128)[:, pair, :])
                ps_ot = [ppo.tile([128, CH], f32, tag="ot", bufs=2,
                                  name=f"ps_ot{_h}")
                         for _h in range(2)]
                njb = 4 * ci + 4
                for jb in range(njb):
                    r = jb - 4 * ci
                    off = 128 * max(r, 0)
                    w = CH - off
                    j0 = jb * 128
                    ps_s = [pps.tile([128, w], f32, tag="s", bufs=3,
                                     name=f"ps_s{_h}")
                            for _h in range(2)]
                    for h in range(2):
                        nc.tensor.matmul(
                            ps_s[h],
                            t_k[64 * h:64 * (h + 1), j0:j0 + 128],
                            t_q[pair][64 * h:64 * (h + 1),
                                      c0 + off:c0 + CH],
                            start=True, stop=(r < 0),
                            tile_position=(64 * h, 0))
                        if r >= 0:
                            # causal mask on the PE: accumulate -1e9
                            # upper-triangle into the diagonal 128 cols
                            # (identity weights x trineg moving). exp
                            # then yields exact zeros -- no DVE dep.
                            nc.tensor.matmul(
                                ps_s[h][:, 0:128], t_id128, t_trineg,
                                start=False, stop=True,
                                skip_group_check=True)
                    for h in range(2):
                        pt = ptp.tile([128, w], bf16, tag="pt", bufs=12)
                        nc.scalar.activation(pt, ps_s[h], EXP)
                        if h == 0:
                            nc.tensor.matmul(
                                ps_ot[h][0:65, off:CH],
                                t_v[jb][:, 0:65], pt,
                                start=(jb == 0), stop=(jb == njb - 1))
                        else:
                            nc.tensor.matmul(
                                ps_ot[h][0:128, off:CH],
                                t_v[jb][:, 64:192], pt,
                                start=(jb == 0), stop=(jb == njb - 1))
                # Stage PSUM -> SBUF fast (frees the ppo banks for the next
                # pair's PV matmuls), then normalize right away: the chain
                # resolves during the next pair / C half-chunk PE work.
                for h in range(2):
                    stage = work.tile([128, CH], f32, tag="otstage",
                                      bufs=6, name=f"stg{pair}{h}")
                    nc.scalar.copy(stage, ps_ot[h])
                    recip = work.tile([1, CH], f32, tag="recip", bufs=4)
                    bcast = work.tile([128, CH], f32, tag="bcast", bufs=4)
                    l_row = stage[64:65, :] if h == 0 else stage[0:1, :]
                    nc.vector.reciprocal(recip, l_row)
                    nc.gpsimd.partition_broadcast(bcast, recip)
                    r0_, r1_ = (0, 64) if h == 0 else (64, 128)
                    nc.vector.tensor_mul(
                        t_ot[pair][r0_:r1_, c0:c0 + CH],
                        stage[r0_:r1_, :], bcast[r0_:r1_, :])

            def emit_C(ci):
                """Partial output projection for this chunk's s-columns.
                mc handled two-at-a-time so each OT weight load serves two
                matmuls (halves LDWEIGHTS traffic)."""
                ci, half = ci
                last = ci == NCH - 1
                for st in range(4 * ci + 2 * half, 4 * ci + 2 * half + 2):
                    t_y = work.tile([128, HIDDEN], bf16, tag="ybounce",
                                    bufs=3)
                    for mc0 in range(0, HIDDEN // CH, 2):
                        ps_y = [ppv.tile([128, CH], f32, tag="misc", bufs=2,
                                         name=f"ps_y{st}_{mc0}_{_j}")
                                for _j in range(2)]
                        for k in range(2):
                            for j in range(2):
                                mc = mc0 + j
                                nc.tensor.matmul(
                                    ps_y[j],
                                    t_ot[k][:, st * 128:(st + 1) * 128],
                                    t_wo[:, k, mc * CH:(mc + 1) * CH],
                                    start=(k == 0), stop=(k == 1))
                        for j in range(2):
                            mc = mc0 + j
                            nc.scalar.copy(
                                t_y[:, mc * CH:(mc + 1) * CH], ps_y[j])
                        if last:
                            # drain the tail chunk in halves so the final
                            # DMA overlaps the remaining matmuls
                            nc.sync.dma_start(
                                out=y[st * 128:(st + 1) * 128,
                                      mc0 * CH:(mc0 + 2) * CH],
                                in_=t_y[:, mc0 * CH:(mc0 + 2) * CH])
                    if not last:
                        nc.sync.dma_start(
                            out=y[st * 128:(st + 1) * 128, :], in_=t_y)

            # Software-pipelined emission: the PE chews on chunk ci+1's
            # projection matmuls while chunk ci's RoPE (ACT/DMA/DVE chain)
            # resolves, so attention never stalls the in-order PE stream.
            t_wo = wpool.tile([128, 2, HIDDEN], bf16, tag="wo")
            for rep in range(repeat):
                # Two-deep software pipeline: PE never waits on the RoPE or
                # normalization chains -- chunk ci+1's projections hide RoPE,
                # and C(ci-1) (dependency-free by then) hides normalization.
                emit_A(0)
                for ci in range(NCH):
                    if ci + 1 < NCH:
                        emit_A(ci + 1)
                    emit_B_pair(ci, 0)
                    if ci - 1 >= 0:
                        emit_C((ci - 1, 0))
                    emit_B_pair(ci, 1)
                    if ci - 1 >= 0:
                        emit_C((ci - 1, 1))
                emit_C((NCH - 1, 0))
                emit_C((NCH - 1, 1))

    nc.compile()
    return nc


def _host_inputs(hidden_states, Wq, Wk, Wv, Wo):
    import ml_dtypes
    bf = ml_dtypes.bfloat16

    hid = np.ascontiguousarray(hidden_states.reshape(S, HIDDEN),
                               dtype=np.float32)
    hT = np.ascontiguousarray(hid.T).astype(bf)

    scale = HD ** -0.5
    inv = 1.0 / (10000.0 ** (np.arange(0, HD, 2, dtype=np.float64) / HD))
    t = np.arange(S, dtype=np.float64)
    freqs = np.outer(t, inv)                       # [S, 32]
    cos_sd = np.repeat(np.cos(freqs), 2, axis=1)   # [S, 64]
    sin_sd = np.repeat(np.sin(freqs), 2, axis=1)
    sign = np.tile(np.array([-1.0, 1.0]), HD // 2)
    cosT = cos_sd.T                                # [64, S]
    sinT = (sin_sd * sign).T
    cosk = np.concatenate([cosT, cosT], 0).astype(np.float32)
    sink = np.concatenate([sinT, sinT], 0).astype(np.float32)
    cosq = (cosk * scale).astype(np.float32)
    sinq = (sink * scale).astype(np.float32)

    tabs = np.zeros((128, NCH, 4 * CH), np.float32)
    for ci in range(NCH):
        sl = slice(ci * CH, (ci + 1) * CH)
        tabs[:, ci, 0:CH] = cosk[:, sl]
        tabs[:, ci, CH:2 * CH] = sink[:, sl]
        tabs[:, ci, 2 * CH:3 * CH] = cosq[:, sl]
        tabs[:, ci, 3 * CH:4 * CH] = sinq[:, sl]

    # trineg[j, i] = -1e9 where j > i (invalid causal), else 0
    trineg = np.tril(np.full((128, 128), -1e9, np.float32), -1).astype(bf)
    ident128 = np.eye(128, dtype=np.float32).astype(bf)
    onespad = np.zeros((128, 64), np.float32)
    onespad[:, 0] = 1.0
    onespad = onespad.astype(bf)
    ident2 = np.tile(np.eye(HD, dtype=np.float32), (2, 1)).astype(bf)

    in_maps = []
    for c in range(N_CORES):
        wq_c = Wq[DQ * c:DQ * (c + 1), :]          # [256, H]
        wk_c = Wk[HD * c:HD * (c + 1), :]          # [64, H]
        wv_c = Wv[HD * c:HD * (c + 1), :]
        wqkvT = np.ascontiguousarray(
            np.concatenate([wq_c, wk_c, wv_c], axis=0).T.astype(np.float32)
        ).astype(bf)
        woT = np.ascontiguousarray(
            Wo[:, DQ * c:DQ * (c + 1)].T.astype(np.float32)).astype(bf)
        in_maps.append({
            "hT": hT, "wqkvT": wqkvT, "woT": woT,
            "tables": tabs,
            "trineg": trineg, "ident128": ident128,
            "onespad": onespad, "ident2": ident2,
        })
    return in_maps


def kernel(hidden_states, Wq, Wk, Wv, Wo):
    from concourse.bass_utils import run_bass_kernel_spmd

    if "nc" not in _cache:
        _cache["nc"] = _build_program()
    nc = _cache["nc"]

    in_maps = _host_inputs(hidden_states, Wq, Wk, Wv, Wo)
    res = run_bass_kernel_spmd(nc, in_maps, list(range(N_CORES)))

    y = np.zeros((S, HIDDEN), np.float64)
    for c in range(N_CORES):
        y += res.results[c]["y"].astype(np.float64)
    return y.astype(np.float32).reshape(1, S, HIDDEN)
